# revision 20
# baseline (speedup 1.0000x reference)
"""BFPLinear Trainium2 kernel.

Computes: out = bfp_quantize(x) @ bfp_quantize(w).T + 2*bias
where bfp_quantize is 8-bit block-floating-point with shared-exponent
groups of 32 along the last (in_features) dim.

Sharding across 8 NeuronCores: 4 batch-groups x 2 column-groups.
Each core gets x[2048, 4096], w[2048, 4096], bias2[2048] and produces
out[2048, 2048].

Quantization is 3 passes over the data:
  1. grouped abs-max reduce (groups of 32 along free dim)
  2. t = x + C  where C = 1.5*2^23 * step encodes round-to-step:
     C_bits = (gmax_bits & 0x7F800000) + 0x08C00000
     fp32 RNE at ulp(t) = step rounds x to the step grid.
  3. xq_bf16 = t - C  (exact; |q| <= 128 is bf16-exact)
The reference clips q to +-127; values that round to +-128 differ by
one step -- ~1e-3 relative error at the output scale, well in tol.

Structure ("level-pipelined"): the contraction is split into two
k-levels (kt 0..15 / 16..31).  Quantized x strips are stored to DRAM
in bf16 natural layout; each level transpose-loads its k-half per
strip and accumulates into the output (level 0 writes o, level 1 adds
via SWDGE accumulate-DMA).  Level 0 over all 16 strips only needs the
first half of the weight quantization, so the PE starts ~3x earlier
than a fully-resident-xqT schedule and W's second half quantizes in
the shadow of level 0.
"""

import os
import numpy as np

import concourse.bass as bass
import concourse.bacc as bacc
import concourse.tile as tile
import concourse.mybir as mybir
from concourse.bass_utils import run_bass_kernel_spmd

F32 = mybir.dt.float32
BF16 = mybir.dt.bfloat16
U32 = mybir.dt.uint32
ALU = mybir.AluOpType
AX = mybir.AxisListType

# Full problem
B_FULL, IN_FULL, OUT_FULL = 8192, 4096, 4096
NBATCH, NCOL = 4, 2  # 4 batch-groups x 2 col-groups = 8 cores
SM_FULL = B_FULL // NBATCH    # 2048 rows of x per core
SN_FULL = OUT_FULL // NCOL    # 2048 output cols per core

GROUP = 32
# C_bits = gexp_bits + (17 << 23) + 0x00400000 (mantissa 1.5)
C_ADD = 0x08C00000


def _bcast_group(t_ap, g, e=GROUP):
    """View a [128, g] tile as [128, g, e] with the inner dim broadcast."""
    return bass.AP(
        tensor=t_ap.tensor,
        offset=t_ap.offset,
        ap=[t_ap.ap[0], t_ap.ap[1], [0, e]],
    )


def build_bass(SM=SM_FULL, SN=SN_FULL, K=IN_FULL, CH=2048):
    """Build the per-core Bass program.

    SM: rows of x shard; SN: rows of w shard (= output cols);
    K: contraction dim; CH: quantization chunk size == k-level size.
    """
    assert K % CH == 0 and CH % 128 == 0 and CH % GROUP == 0
    assert K // CH == 2, "emission interleave assumes two k-levels"
    NKT = K // 128          # k-tiles
    CHT = CH // 128         # k-tiles per chunk (= per level)
    G = CH // GROUP         # groups per chunk
    NCH = K // CH           # chunks per strip = number of k-levels
    MS = SM // 128          # m-strips
    NS = SN // 128          # w row strips
    NSL = (SN + 511) // 512  # 512-wide n slices per psum

    nc = bacc.Bacc("TRN2", target_bir_lowering=False)

    x = nc.dram_tensor("x", [SM, K], F32, kind="ExternalInput")
    w = nc.dram_tensor("w", [SN, K], F32, kind="ExternalInput")
    b2 = nc.dram_tensor("b2", [SN], F32, kind="ExternalInput")
    o = nc.dram_tensor("o", [SM, SN], F32, kind="ExternalOutput")

    with tile.TileContext(nc) as tc:
        with (
            tc.tile_pool(name="res", bufs=1) as res_p,
            tc.tile_pool(name="nat", bufs=4) as nat_p,
            tc.tile_pool(name="qb", bufs=3) as qb_p,
            tc.tile_pool(name="tiny", bufs=8) as tiny_p,
            tc.tile_pool(name="xqt", bufs=3) as xqt_p,
            tc.tile_pool(name="outp", bufs=2) as out_p,
            tc.tile_pool(name="psum", bufs=2, space="PSUM") as psum_p,
            tc.tile_pool(name="dram", bufs=1, space="DRAM") as dr_p,
        ):
            # resident quantized-transposed weights, one tile per
            # 512-wide n block (== matmul nj slice)
            wqT = [res_p.tile([128, NKT, 512], BF16, tag=f"wq{j}",
                              name=f"wq{j}")
                   for j in range(NSL)]
            # quantized x, bf16 natural layout, staged in DRAM per
            # (strip, level); transpose-loaded by each level pass
            xq = dr_p.tile([MS * NCH * 128, CH], BF16)

            def xq_rows(m, h):
                r0 = (m * NCH + h) * 128
                return xq[r0:r0 + 128, :]
            # bias row (bf16) + ones column for the PE bias-accumulate
            bias2b = res_p.tile([1, SN], BF16)
            ones = res_p.tile([1, 128], BF16)
            nc.gpsimd.dma_start(
                out=bias2b,
                in_=bass.AP(tensor=b2, offset=0, ap=[[0, 1], [1, SN]]),
            )
            nc.vector.memset(ones, 1.0)

            def quant_stage_a(src_slice):
                """Load one [128, CH] fp32 chunk, grouped abs-max, build C.
                Returns (nat, gmax) for stage B."""
                nat = nat_p.tile([128, CH], F32, tag="nat")
                nc.sync.dma_start(out=nat, in_=src_slice)
                nat3 = nat[:].rearrange("p (g e) -> p g e", e=GROUP)

                gmax = tiny_p.tile([128, G], F32, tag="gmax")
                nc.vector.tensor_reduce(
                    out=gmax[:], in_=nat3, axis=AX.X, op=ALU.max,
                    apply_absolute_value=True,
                )
                # C = 1.5 * 2^23 * step, built from the exponent bits
                # (walrus rejects bitwise+arith in one dual-op ts)
                nc.vector.tensor_scalar(
                    out=gmax[:].bitcast(U32), in0=gmax[:].bitcast(U32),
                    scalar1=0x7F800000, scalar2=None, op0=ALU.bitwise_and,
                )
                nc.vector.tensor_scalar(
                    out=gmax[:].bitcast(U32), in0=gmax[:].bitcast(U32),
                    scalar1=C_ADD, scalar2=None, op0=ALU.add,
                )
                return nat, gmax

            def quant_stage_b(nat, gmax, add_eng):
                """t = x + C (in-place), xq = t - C -> bf16.
                Returns the bf16 qb tile."""
                nat3 = nat[:].rearrange("p (g e) -> p g e", e=GROUP)
                cb = _bcast_group(gmax[:], G)
                add_eng.tensor_tensor(out=nat3, in0=nat3, in1=cb,
                                      op=ALU.add)
                qb = qb_p.tile([128, CH], BF16, tag="qb")
                qb3 = qb[:].rearrange("p (g e) -> p g e", e=GROUP)
                nc.vector.tensor_tensor(out=qb3, in0=nat3, in1=cb,
                                        op=ALU.subtract)
                return qb

            # ---- W quantization: h-major so level-0 matmuls only wait
            # for the h=0 half; transposed directly into resident wqT
            def quant_w_batch(h, s_list):
                staged = [(quant_stage_a(
                    w[s * 128:(s + 1) * 128, h * CH:(h + 1) * CH]), s)
                    for s in s_list]
                for (nat, gmax), s in staged:
                    qb = quant_stage_b(nat, gmax, nc.gpsimd)
                    nj, no = divmod(s * 128, 512)
                    nc.scalar.dma_start_transpose(
                        out=wqT[nj][:, h * CHT:(h + 1) * CHT, no:no + 128],
                        in_=qb[:],
                    )

            # ---- X quantization: store bf16 chunks to DRAM
            def quant_x_batch(jobs):
                staged = [(quant_stage_a(
                    x[m * 128:(m + 1) * 128, h * CH:(h + 1) * CH]), m, h)
                    for m, h in jobs]
                for (nat, gmax), m, h in staged:
                    qb = quant_stage_b(nat, gmax, nc.gpsimd)
                    nc.sync.dma_start(out=xq_rows(m, h), in_=qb[:])

            WB = 4
            # W h=0 first (gates level 0), then a few x strips, then W
            # h=1 (gates level 1, hidden under level 0), then rest of x.
            for s0 in range(0, NS, WB):
                quant_w_batch(0, range(s0, min(NS, s0 + WB)))
            quant_x_batch([(m, h) for m in range(0, min(4, MS))
                           for h in range(NCH)])
            for s0 in range(0, NS, WB):
                quant_w_batch(1, range(s0, min(NS, s0 + WB)))
            for m0 in range(4, MS, WB):
                quant_x_batch([(m, h)
                               for m in range(m0, min(MS, m0 + WB))
                               for h in range(NCH)])

            # ---- level passes: transpose-load xq half, matmul, evict
            xqt_tiles = {}

            def load_xqt(m, h):
                xqt = xqt_p.tile([128, CHT, 128], BF16, tag="xqt")
                xqt_tiles[(m, h)] = xqt
                nc.scalar.dma_start_transpose(
                    out=xqt[:], in_=xq_rows(m, h),
                )

            for h in range(NCH):
                load_xqt(0, h)
                if MS > 1:
                    load_xqt(1, h)
                for m in range(MS):
                    if m + 2 < MS:
                        load_xqt(m + 2, h)
                    xqt = xqt_tiles.pop((m, h))
                    psum = psum_p.tile([128, SN], F32, tag="psum")
                    if h == 0:
                        for nj in range(NSL):
                            n0 = nj * 512
                            # seed PSUM with the (doubled) bias
                            nc.tensor.matmul(
                                psum[:, n0:n0 + 512],
                                ones[:],
                                bias2b[:, n0:n0 + 512],
                                start=True,
                                stop=False,
                            )
                    for kk in range(CHT):
                        kt = h * CHT + kk
                        for nj in range(NSL):
                            n0 = nj * 512
                            nc.tensor.matmul(
                                psum[:, n0:n0 + 512],
                                xqt[:, kk, :],
                                wqT[nj][:, kt, :],
                                start=(h != 0 and kk == 0),
                                stop=(kk == CHT - 1),
                            )
                    outt = out_p.tile([128, SN], F32, tag="outt")
                    nc.scalar.copy(out=outt[:], in_=psum[:])
                    if h == 0:
                        nc.sync.dma_start(
                            out=o[m * 128:(m + 1) * 128, :], in_=outt[:]
                        )
                    else:
                        # accumulate the second k-level into the output
                        nc.gpsimd.dma_start(
                            out=o[m * 128:(m + 1) * 128, :], in_=outt[:],
                            accum_op=ALU.add,
                        )

    nc.compile()
    return nc


_NC_CACHE = {}


def _get_nc(key=("full",)):
    if key not in _NC_CACHE:
        if key == ("full",):
            _NC_CACHE[key] = build_bass()
        else:
            _NC_CACHE[key] = build_bass(*key)
    return _NC_CACHE[key]


def kernel(input, weight, bias):
    input = np.ascontiguousarray(input, dtype=np.float32)
    weight = np.ascontiguousarray(weight, dtype=np.float32)
    bias = np.ascontiguousarray(bias, dtype=np.float32)

    nc = _get_nc()
    b2_full = bias * np.float32(2.0)

    in_maps = []
    for c in range(8):
        bi, ni = divmod(c, NCOL)
        in_maps.append({
            "x": input[bi * SM_FULL:(bi + 1) * SM_FULL, :],
            "w": weight[ni * SN_FULL:(ni + 1) * SN_FULL, :],
            "b2": b2_full[ni * SN_FULL:(ni + 1) * SN_FULL],
        })

    trace = bool(int(os.environ.get("BFP_TRACE", "0")))
    res = run_bass_kernel_spmd(
        nc, in_maps, core_ids=list(range(8)), trace=trace,
    )
    kernel.last_results = res

    out = np.empty((B_FULL, OUT_FULL), dtype=np.float32)
    for c in range(8):
        bi, ni = divmod(c, NCOL)
        out[bi * SM_FULL:(bi + 1) * SM_FULL,
            ni * SN_FULL:(ni + 1) * SN_FULL] = res.results[c]["o"]
    return out


def bench(ins, iters=6):
    """Wall-clock timing (axon PJRT transfer dominates; the trace path
    gives true device time)."""
    import time

    t0 = time.perf_counter()
    kernel(**ins)
    dt = time.perf_counter() - t0
    print("bench[wall incl gather]: %.3f ms" % (dt * 1e3))
    return max(1, int(dt * 1e9))


if __name__ == "__main__":
    import sys
    mode = sys.argv[1] if len(sys.argv) > 1 else "sim"
    if mode == "sim":
        # quick numerical validation in CoreSim on a small config
        from concourse.bass_interp import CoreSim
        SM, SN, K, CH = 256, 1024, 512, 256
        nc = build_bass(SM, SN, K, CH)
        rng = np.random.default_rng(0)
        xin = rng.standard_normal((SM, K), dtype=np.float32)
        win = rng.uniform(-0.1, 0.1, (SN, K)).astype(np.float32)
        bin_ = rng.uniform(-0.1, 0.1, SN).astype(np.float32)

        sim = CoreSim(nc)
        sim.tensor("x")[:] = xin
        sim.tensor("w")[:] = win
        sim.tensor("b2")[:] = bin_ * 2.0
        sim.simulate(check_with_hw=False)
        got = np.array(sim.tensor("o"))

        def bfpq(v):
            g = v.reshape(v.shape[0], -1, GROUP).astype(np.float64)
            ma = np.abs(g).max(axis=-1, keepdims=True)
            e = np.floor(np.log2(np.where(ma > 0, ma, 1.0)))
            st = np.exp2(e - 6)
            qq = np.clip(np.round(g / st), -127, 127) * st
            return np.where(ma > 0, qq, 0.0).reshape(v.shape)

        exp = bfpq(xin) @ bfpq(win).T + 2.0 * bin_.astype(np.float64)
        err = np.abs(got.astype(np.float64) - exp)
        rel = err.max() / np.abs(exp).max()
        print("max abs err:", err.max(), "rel:", rel)
        assert rel < 1e-3, "numerical mismatch"
        print("SIM PASS")
    elif mode == "hw":
        import reference
        ins = {k: np.asarray(v) for k, v in reference.setup_inputs().items()}
        outp = kernel(**ins)
        print("out", outp.shape, outp.dtype)


# revision 24
# speedup vs baseline: 1.1267x; 1.1267x over previous
"""BFPLinear Trainium2 kernel.

Computes: out = bfp_quantize(x) @ bfp_quantize(w).T + 2*bias
where bfp_quantize is 8-bit block-floating-point with shared-exponent
groups of 32 along the last (in_features) dim.

Sharding across 8 NeuronCores: 4 batch-groups x 2 column-groups.
Each core gets x[2048, 4096] and produces out[2048, 2048].  The w
shard [2048, 4096] of each column-group is quantized cooperatively:
each of the 4 batch-replicas quantizes+transposes a 512-row quarter
(fed as the `wp` input) and the quarters are AllGathered (groups
{0,2,4,6} / {1,3,5,7}) in transposed bf16 layout.

Quantization is 3 passes over the data:
  1. grouped abs-max reduce (groups of 32 along free dim)
  2. t = x + C  where C = 1.5*2^23 * step encodes round-to-step:
     C_bits = (gmax_bits & 0x7F800000) + 0x08C00000
     fp32 RNE at ulp(t) = step rounds x to the step grid.
  3. xq_bf16 = t - C  (exact; |q| <= 128 is bf16-exact)
The reference clips q to +-127; values that round to +-128 differ by
one step -- ~1e-3 relative error at the output scale, well in tol.

Steady-state x-strip quant runs DVE-only (no DVE/GPSIMD SBUF-port
contention); W-phase adds run on GPSIMD; PSUM eviction is a pure ACT
copy (bias is pre-seeded into PSUM via a K=1 ones x bias matmul).
"""

import os
import numpy as np

import concourse.bass as bass
import concourse.bacc as bacc
import concourse.tile as tile
import concourse.mybir as mybir
from concourse.bass_utils import run_bass_kernel_spmd

F32 = mybir.dt.float32
BF16 = mybir.dt.bfloat16
U32 = mybir.dt.uint32
ALU = mybir.AluOpType
AX = mybir.AxisListType

# Full problem
B_FULL, IN_FULL, OUT_FULL = 8192, 4096, 4096
NBATCH, NCOL = 4, 2  # 4 batch-groups x 2 col-groups = 8 cores
SM_FULL = B_FULL // NBATCH    # 2048 rows of x per core
SN_FULL = OUT_FULL // NCOL    # 2048 output cols per core
WP_ROWS = SN_FULL // NBATCH   # 512 w rows quantized per core

GROUP = 32
# C_bits = gexp_bits + (17 << 23) + 0x00400000 (mantissa 1.5)
C_ADD = 0x08C00000

REPLICA_GROUPS = [[0, 1, 2, 3, 4, 5, 6, 7]]


def _bcast_group(t_ap, g, e=GROUP):
    """View a [128, g] tile as [128, g, e] with the inner dim broadcast."""
    return bass.AP(
        tensor=t_ap.tensor,
        offset=t_ap.offset,
        ap=[t_ap.ap[0], t_ap.ap[1], [0, e]],
    )


def build_bass(SM=SM_FULL, SN=SN_FULL, K=IN_FULL, CH=2048, ag=True):
    """Build the per-core Bass program.

    SM: rows of x shard; SN: output cols per core; K: contraction dim;
    CH: quantization chunk size; ag: use the 4-way w-quantization
    AllGather (ag=False quantizes the full w shard locally -- used for
    single-core CoreSim validation).
    """
    assert K % CH == 0 and CH % 128 == 0 and CH % GROUP == 0
    NKT = K // 128          # k-tiles
    CHT = CH // 128         # k-tiles per chunk
    G = CH // GROUP         # groups per chunk
    NCH = K // CH           # chunks per strip
    MS = SM // 128          # m-strips
    NS = SN // 128          # w row strips (total, ag=False path)
    NSL = (SN + 511) // 512  # 512-wide n slices per psum
    WPS = (SN // NBATCH) // 128 if ag else None  # w strips per core (ag)

    nc = bacc.Bacc("TRN2", target_bir_lowering=False, num_devices=8)

    x = nc.dram_tensor("x", [SM, K], F32, kind="ExternalInput")
    if ag:
        wp = nc.dram_tensor("wp", [SN // NBATCH, K], F32,
                            kind="ExternalInput")
    else:
        w = nc.dram_tensor("w", [SN, K], F32, kind="ExternalInput")
    b2 = nc.dram_tensor("b2", [SN], F32, kind="ExternalInput")
    o = nc.dram_tensor("o", [SM, SN], F32, kind="ExternalOutput")

    with tile.TileContext(nc) as tc:
        with (
            tc.tile_pool(name="res", bufs=1) as res_p,
            tc.tile_pool(name="nat", bufs=4) as nat_p,
            tc.tile_pool(name="qb", bufs=2) as qb_p,
            tc.tile_pool(name="tiny", bufs=8) as tiny_p,
            tc.tile_pool(name="stg", bufs=2) as stg_p,
            tc.tile_pool(name="xqt", bufs=2) as xqt_p,
            tc.tile_pool(name="outp", bufs=1) as out_p,
            tc.tile_pool(name="psum", bufs=2, space="PSUM") as psum_p,
            tc.tile_pool(name="dram", bufs=1, space="DRAM") as dr_p,
        ):
            # resident quantized-transposed weights, one tile per
            # 512-wide n block (== matmul nj slice == AG block)
            wqT = [res_p.tile([128, NKT, 512], BF16, tag=f"wq{j}",
                              name=f"wq{j}")
                   for j in range(NSL)]
            # bias row (bf16) + ones column for the PE bias-accumulate
            bias2b = res_p.tile([1, SN], BF16)
            ones = res_p.tile([1, 128], BF16)
            nc.gpsimd.dma_start(
                out=bias2b,
                in_=bass.AP(tensor=b2, offset=0, ap=[[0, 1], [1, SN]]),
            )
            nc.vector.memset(ones, 1.0)

            def quant_stage_a(src_slice):
                """Load one [128, CH] fp32 chunk, grouped abs-max, build C.
                Returns (nat, gmax) for stage B."""
                nat = nat_p.tile([128, CH], F32, tag="nat")
                nc.sync.dma_start(out=nat, in_=src_slice)
                nat3 = nat[:].rearrange("p (g e) -> p g e", e=GROUP)

                gmax = tiny_p.tile([128, G], F32, tag="gmax")
                nc.vector.tensor_reduce(
                    out=gmax[:], in_=nat3, axis=AX.X, op=ALU.max,
                    apply_absolute_value=True,
                )
                # C = 1.5 * 2^23 * step, built from the exponent bits
                # (walrus rejects bitwise+arith in one dual-op ts)
                nc.vector.tensor_scalar(
                    out=gmax[:].bitcast(U32), in0=gmax[:].bitcast(U32),
                    scalar1=0x7F800000, scalar2=None, op0=ALU.bitwise_and,
                )
                nc.vector.tensor_scalar(
                    out=gmax[:].bitcast(U32), in0=gmax[:].bitcast(U32),
                    scalar1=C_ADD, scalar2=None, op0=ALU.add,
                )
                return nat, gmax

            def quant_stage_b(nat, gmax, dst3d, add_eng):
                """t = x + C (in-place), xq = t - C -> bf16, then
                xbar-transpose into dst3d ([128, CHT, 128])."""
                nat3 = nat[:].rearrange("p (g e) -> p g e", e=GROUP)
                cb = _bcast_group(gmax[:], G)
                add_eng.tensor_tensor(out=nat3, in0=nat3, in1=cb,
                                      op=ALU.add)
                qb = qb_p.tile([128, CH], BF16, tag="qb")
                qb3 = qb[:].rearrange("p (g e) -> p g e", e=GROUP)
                nc.vector.tensor_tensor(out=qb3, in0=nat3, in1=cb,
                                        op=ALU.subtract)
                # blocked xbar transpose: [128, CH] -> [128, CHT, 128]
                nc.scalar.dma_start_transpose(out=dst3d, in_=qb[:])

            def quant_batch(jobs, add_eng):
                """Software-pipelined quantization of a batch of
                (src_slice, dst3d) chunk jobs: all A stages, then all B
                stages, so each engine streams without ping-pong stalls."""
                staged = [(quant_stage_a(src), dst) for src, dst in jobs]
                for (nat, gmax), dst in staged:
                    quant_stage_b(nat, gmax, dst, add_eng)

            xqt_tiles = {}

            def quant_x_strip(m, add_eng):
                xqt = xqt_p.tile([128, NKT, 128], BF16, tag="xqt")
                xqt_tiles[m] = xqt
                quant_batch([
                    (x[m * 128:(m + 1) * 128, h * CH:(h + 1) * CH],
                     xqt[:, h * CHT:(h + 1) * CHT, :])
                    for h in range(NCH)
                ], add_eng)

            # ---- W phase ------------------------------------------------
            if ag:
                # each core quantizes+transposes its 512-row quarter of
                # the column-group's w shard, stores it to DRAM, and the
                # four batch-replicas AllGather the transposed parts.
                cc_in = [dr_p.tile([128, CHT, 512], BF16,
                                   name=f"ccin{h}", tag=f"ccin{h}")
                         for h in range(NCH)]
                cc_out = [dr_p.tile([1024, CHT, 512], BF16,
                                    name=f"ccout{h}", tag=f"ccout{h}",
                                    addr_space="Shared")
                          for h in range(NCH)]
                for h in range(NCH):
                    staged = []
                    for sl in range(WPS):
                        nat, gmax = quant_stage_a(
                            wp[sl * 128:(sl + 1) * 128,
                               h * CH:(h + 1) * CH])
                        staged.append((nat, gmax, sl))
                    for nat, gmax, sl in staged:
                        stg = stg_p.tile([128, CHT, 128], BF16, tag="stg")
                        quant_stage_b(nat, gmax, stg[:], nc.gpsimd)
                        nc.sync.dma_start(
                            out=cc_in[h][:, :, sl * 128:(sl + 1) * 128],
                            in_=stg[:],
                        )
                # quantize a few x strips before the AG-dependent
                # readbacks so their transposes aren't stuck behind the
                # AG wait in the scalar HWDGE FIFO
                quant_x_strip(0, nc.vector)
                quant_x_strip(1, nc.vector)
                for h in range(NCH):
                    nc.gpsimd.collective_compute(
                        "AllGather",
                        ALU.bypass,
                        ins=[cc_in[h][:, :, :]],
                        outs=[cc_out[h][:, :, :]],
                        replica_groups=REPLICA_GROUPS,
                    )
                # 8-core gather interleaves the two column-groups:
                # block index of logical block j on this core is
                # 2*j + (core % 2); readback picks our 4 blocks via a
                # partition-id-derived dynamic DRAM offset.
                ni_sv = nc.scalar.partition_id() % 2
                BLK = 128 * CHT * 512

                def read_block(h, j):
                    base = cc_out[h][:]
                    in_ap = bass.AP(
                        tensor=base.tensor,
                        offset=base.offset + 2 * j * BLK + ni_sv * BLK,
                        ap=[[CHT * 512, 128], [512, CHT], [1, 512]],
                    )
                    nc.scalar.dma_start(
                        out=wqT[j][:, h * CHT:(h + 1) * CHT, :],
                        in_=in_ap,
                    )

                for j in range(NSL):
                    read_block(0, j)
                for j in range(NSL):
                    read_block(1, j)
            else:
                WB = 4
                for h in range(NCH):
                    for s0 in range(0, NS, WB):
                        for s in range(s0, min(NS, s0 + WB)):
                            nat, gmax = quant_stage_a(
                                w[s * 128:(s + 1) * 128,
                                  h * CH:(h + 1) * CH])
                            nj, no = divmod(s * 128, 512)
                            quant_stage_b(
                                nat, gmax,
                                wqT[nj][:, h * CHT:(h + 1) * CHT,
                                        no:no + 128],
                                nc.gpsimd,
                            )

            # ---- X phase helpers (used by both W-phase prefetch and
            # the steady-state loop).  Steady-state x quant runs
            # DVE-only so the two vector engines never contend.
            if not ag:
                quant_x_strip(0, nc.vector)
                quant_x_strip(1, nc.vector)

            for m in range(MS):
                if m + 2 < MS and (m + 2) not in xqt_tiles:
                    quant_x_strip(m + 2, nc.vector)
                xqt = xqt_tiles[m]
                psum = psum_p.tile([128, SN], F32, tag="psum")
                for nj in range(NSL):
                    n0 = nj * 512
                    n1 = min(SN, n0 + 512)
                    # seed PSUM with the (doubled) bias via a K=1 matmul
                    nc.tensor.matmul(
                        psum[:, n0:n1],
                        ones[:],
                        bias2b[:, n0:n1],
                        start=True,
                        stop=False,
                    )
                for kt in range(NKT):
                    for nj in range(NSL):
                        n0 = nj * 512
                        n1 = min(SN, n0 + 512)
                        nc.tensor.matmul(
                            psum[:, n0:n1],
                            xqt[:, kt, :],
                            wqT[nj][:, kt, :n1 - n0],
                            start=False,
                            stop=(kt == NKT - 1),
                        )
                outt = out_p.tile([128, SN], F32, tag="outt")
                nc.scalar.copy(out=outt[:], in_=psum[:])
                nc.sync.dma_start(
                    out=o[m * 128:(m + 1) * 128, :], in_=outt[:]
                )

    nc.compile()
    return nc


_NC_CACHE = {}


def _get_nc(key=("full",)):
    if key not in _NC_CACHE:
        if key == ("full",):
            _NC_CACHE[key] = build_bass()
        else:
            _NC_CACHE[key] = build_bass(*key)
    return _NC_CACHE[key]


def kernel(input, weight, bias):
    input = np.ascontiguousarray(input, dtype=np.float32)
    weight = np.ascontiguousarray(weight, dtype=np.float32)
    bias = np.ascontiguousarray(bias, dtype=np.float32)

    nc = _get_nc()
    b2_full = bias * np.float32(2.0)

    in_maps = []
    for c in range(8):
        bi, ni = divmod(c, NCOL)
        w0 = ni * SN_FULL + bi * WP_ROWS
        in_maps.append({
            "x": input[bi * SM_FULL:(bi + 1) * SM_FULL, :],
            "wp": weight[w0:w0 + WP_ROWS, :],
            "b2": b2_full[ni * SN_FULL:(ni + 1) * SN_FULL],
        })

    trace = bool(int(os.environ.get("BFP_TRACE", "0")))
    res = run_bass_kernel_spmd(
        nc, in_maps, core_ids=list(range(8)), trace=trace,
    )
    kernel.last_results = res

    out = np.empty((B_FULL, OUT_FULL), dtype=np.float32)
    for c in range(8):
        bi, ni = divmod(c, NCOL)
        out[bi * SM_FULL:(bi + 1) * SM_FULL,
            ni * SN_FULL:(ni + 1) * SN_FULL] = res.results[c]["o"]
    return out


def bench(ins, iters=6):
    """Wall-clock timing of the jitted 8-core kernel (axon PJRT
    round-trip dominates; trace path gives true device time)."""
    import time
    import jax

    t0 = time.perf_counter()
    out = kernel(**ins)
    dt = time.perf_counter() - t0
    print("bench[wall incl gather]: %.3f ms" % (dt * 1e3))
    return max(1, int(dt * 1e9))


if __name__ == "__main__":
    import sys
    mode = sys.argv[1] if len(sys.argv) > 1 else "sim"
    if mode == "sim":
        # quick numerical validation in CoreSim on a small config
        # (ag=False: single-core local-W path exercises the same quant
        # math and matmul structure)
        from concourse.bass_interp import CoreSim
        SM, SN, K, CH = 256, 1024, 512, 256
        nc = build_bass(SM, SN, K, CH, ag=False)
        rng = np.random.default_rng(0)
        xin = rng.standard_normal((SM, K), dtype=np.float32)
        win = rng.uniform(-0.1, 0.1, (SN, K)).astype(np.float32)
        bin_ = rng.uniform(-0.1, 0.1, SN).astype(np.float32)

        sim = CoreSim(nc)
        sim.tensor("x")[:] = xin
        sim.tensor("w")[:] = win
        sim.tensor("b2")[:] = bin_ * 2.0
        sim.simulate(check_with_hw=False)
        got = np.array(sim.tensor("o"))

        def bfpq(v):
            g = v.reshape(v.shape[0], -1, GROUP).astype(np.float64)
            ma = np.abs(g).max(axis=-1, keepdims=True)
            e = np.floor(np.log2(np.where(ma > 0, ma, 1.0)))
            st = np.exp2(e - 6)
            qq = np.clip(np.round(g / st), -127, 127) * st
            return np.where(ma > 0, qq, 0.0).reshape(v.shape)

        exp = bfpq(xin) @ bfpq(win).T + 2.0 * bin_.astype(np.float64)
        err = np.abs(got.astype(np.float64) - exp)
        rel = err.max() / np.abs(exp).max()
        print("max abs err:", err.max(), "rel:", rel)
        assert rel < 1e-3, "numerical mismatch"
        print("SIM PASS")
    elif mode == "hw":
        import reference
        ins = {k: np.asarray(v) for k, v in reference.setup_inputs().items()}
        outp = kernel(**ins)
        print("out", outp.shape, outp.dtype)


# revision 25
# speedup vs baseline: 1.3257x; 1.1766x over previous
"""BFPLinear Trainium2 kernel.

Computes: out = bfp_quantize(x) @ bfp_quantize(w).T + 2*bias
where bfp_quantize is 8-bit block-floating-point with shared-exponent
groups of 32 along the last (in_features) dim.

Sharding across 8 NeuronCores: 4 batch-groups x 2 column-groups.
Each core gets x[2048, 4096] and produces out[2048, 2048].  The w
shard [2048, 4096] of each column-group is quantized cooperatively:
each of the 4 batch-replicas quantizes+transposes a 512-row quarter
(fed as the `wp` input) and the quarters are AllGathered (groups
{0,2,4,6} / {1,3,5,7}) in transposed bf16 layout.

Quantization is 3 passes over the data:
  1. grouped abs-max reduce (groups of 32 along free dim)
  2. t = x + C  where C = 1.5*2^23 * step encodes round-to-step:
     C_bits = (gmax_bits & 0x7F800000) + 0x08C00000
     fp32 RNE at ulp(t) = step rounds x to the step grid.
  3. xq_bf16 = t - C  (exact; |q| <= 128 is bf16-exact)
The reference clips q to +-127; values that round to +-128 differ by
one step -- ~1e-3 relative error at the output scale, well in tol.

Steady-state x-strip quant runs DVE-only (no DVE/GPSIMD SBUF-port
contention); W-phase adds run on GPSIMD; PSUM eviction is a pure ACT
copy (bias is pre-seeded into PSUM via a K=1 ones x bias matmul).
"""

import os
import numpy as np

import concourse.bass as bass
import concourse.bacc as bacc
import concourse.tile as tile
import concourse.mybir as mybir
from concourse.bass_utils import run_bass_kernel_spmd

F32 = mybir.dt.float32
BF16 = mybir.dt.bfloat16
U32 = mybir.dt.uint32
ALU = mybir.AluOpType
AX = mybir.AxisListType

# Full problem
B_FULL, IN_FULL, OUT_FULL = 8192, 4096, 4096
NBATCH, NCOL = 4, 2  # 4 batch-groups x 2 col-groups = 8 cores
SM_FULL = B_FULL // NBATCH    # 2048 rows of x per core
SN_FULL = OUT_FULL // NCOL    # 2048 output cols per core
WP_ROWS = SN_FULL // NBATCH   # 512 w rows quantized per core

GROUP = 32
# C_bits = gexp_bits + (17 << 23) + 0x00400000 (mantissa 1.5)
C_ADD = 0x08C00000

REPLICA_GROUPS = [[0, 2, 4, 6], [1, 3, 5, 7]]


def _bcast_group(t_ap, g, e=GROUP):
    """View a [128, g] tile as [128, g, e] with the inner dim broadcast."""
    return bass.AP(
        tensor=t_ap.tensor,
        offset=t_ap.offset,
        ap=[t_ap.ap[0], t_ap.ap[1], [0, e]],
    )


def build_bass(SM=SM_FULL, SN=SN_FULL, K=IN_FULL, CH=2048, ag=True):
    """Build the per-core Bass program.

    SM: rows of x shard; SN: output cols per core; K: contraction dim;
    CH: quantization chunk size; ag: use the 4-way w-quantization
    AllGather (ag=False quantizes the full w shard locally -- used for
    single-core CoreSim validation).
    """
    assert K % CH == 0 and CH % 128 == 0 and CH % GROUP == 0
    NKT = K // 128          # k-tiles
    CHT = CH // 128         # k-tiles per chunk
    G = CH // GROUP         # groups per chunk
    NCH = K // CH           # chunks per strip
    MS = SM // 128          # m-strips
    NS = SN // 128          # w row strips (total, ag=False path)
    NSL = (SN + 511) // 512  # 512-wide n slices per psum
    WPS = (SN // NBATCH) // 128 if ag else None  # w strips per core (ag)

    nc = bacc.Bacc("TRN2", target_bir_lowering=False, num_devices=8)

    x = nc.dram_tensor("x", [SM, K], F32, kind="ExternalInput")
    if ag:
        wp = nc.dram_tensor("wp", [SN // NBATCH, K], F32,
                            kind="ExternalInput")
    else:
        w = nc.dram_tensor("w", [SN, K], F32, kind="ExternalInput")
    b2 = nc.dram_tensor("b2", [SN], F32, kind="ExternalInput")
    o = nc.dram_tensor("o", [SM, SN], F32, kind="ExternalOutput")

    with tile.TileContext(nc) as tc:
        with (
            tc.tile_pool(name="res", bufs=1) as res_p,
            tc.tile_pool(name="nat", bufs=4) as nat_p,
            tc.tile_pool(name="qb", bufs=2) as qb_p,
            tc.tile_pool(name="tiny", bufs=8) as tiny_p,
            tc.tile_pool(name="stg", bufs=2) as stg_p,
            tc.tile_pool(name="xqt", bufs=2) as xqt_p,
            tc.tile_pool(name="outp", bufs=1) as out_p,
            tc.tile_pool(name="psum", bufs=2, space="PSUM") as psum_p,
            tc.tile_pool(name="dram", bufs=1, space="DRAM") as dr_p,
        ):
            # resident quantized-transposed weights, one tile per
            # 512-wide n block (== matmul nj slice == AG block)
            wqT = [res_p.tile([128, NKT, 512], BF16, tag=f"wq{j}",
                              name=f"wq{j}")
                   for j in range(NSL)]
            # bias row (bf16) + ones column for the PE bias-accumulate
            bias2b = res_p.tile([1, SN], BF16)
            ones = res_p.tile([1, 128], BF16)
            nc.gpsimd.dma_start(
                out=bias2b,
                in_=bass.AP(tensor=b2, offset=0, ap=[[0, 1], [1, SN]]),
            )
            nc.vector.memset(ones, 1.0)

            def quant_stage_a(src_slice):
                """Load one [128, CH] fp32 chunk, grouped abs-max, build C.
                Returns (nat, gmax) for stage B."""
                nat = nat_p.tile([128, CH], F32, tag="nat")
                nc.sync.dma_start(out=nat, in_=src_slice)
                nat3 = nat[:].rearrange("p (g e) -> p g e", e=GROUP)

                gmax = tiny_p.tile([128, G], F32, tag="gmax")
                nc.vector.tensor_reduce(
                    out=gmax[:], in_=nat3, axis=AX.X, op=ALU.max,
                    apply_absolute_value=True,
                )
                # C = 1.5 * 2^23 * step, built from the exponent bits
                # (walrus rejects bitwise+arith in one dual-op ts)
                nc.vector.tensor_scalar(
                    out=gmax[:].bitcast(U32), in0=gmax[:].bitcast(U32),
                    scalar1=0x7F800000, scalar2=None, op0=ALU.bitwise_and,
                )
                nc.vector.tensor_scalar(
                    out=gmax[:].bitcast(U32), in0=gmax[:].bitcast(U32),
                    scalar1=C_ADD, scalar2=None, op0=ALU.add,
                )
                return nat, gmax

            def quant_stage_b(nat, gmax, dst3d, add_eng):
                """t = x + C (in-place), xq = t - C -> bf16, then
                xbar-transpose into dst3d ([128, CHT, 128])."""
                nat3 = nat[:].rearrange("p (g e) -> p g e", e=GROUP)
                cb = _bcast_group(gmax[:], G)
                add_eng.tensor_tensor(out=nat3, in0=nat3, in1=cb,
                                      op=ALU.add)
                qb = qb_p.tile([128, CH], BF16, tag="qb")
                qb3 = qb[:].rearrange("p (g e) -> p g e", e=GROUP)
                nc.vector.tensor_tensor(out=qb3, in0=nat3, in1=cb,
                                        op=ALU.subtract)
                # blocked xbar transpose: [128, CH] -> [128, CHT, 128]
                nc.scalar.dma_start_transpose(out=dst3d, in_=qb[:])

            def quant_batch(jobs, add_eng):
                """Software-pipelined quantization of a batch of
                (src_slice, dst3d) chunk jobs: all A stages, then all B
                stages, so each engine streams without ping-pong stalls."""
                staged = [(quant_stage_a(src), dst) for src, dst in jobs]
                for (nat, gmax), dst in staged:
                    quant_stage_b(nat, gmax, dst, add_eng)

            # ---- W phase ------------------------------------------------
            if ag:
                # each core quantizes+transposes its 512-row quarter of
                # the column-group's w shard, stores it to DRAM, and the
                # four batch-replicas AllGather the transposed parts.
                cc_in = [dr_p.tile([128, CHT, 512], BF16,
                                   name=f"ccin{h}", tag=f"ccin{h}")
                         for h in range(NCH)]
                cc_out = [dr_p.tile([512, CHT, 512], BF16,
                                    name=f"ccout{h}", tag=f"ccout{h}")
                          for h in range(NCH)]
                for h in range(NCH):
                    staged = []
                    for sl in range(WPS):
                        nat, gmax = quant_stage_a(
                            wp[sl * 128:(sl + 1) * 128,
                               h * CH:(h + 1) * CH])
                        staged.append((nat, gmax, sl))
                    for nat, gmax, sl in staged:
                        stg = stg_p.tile([128, CHT, 128], BF16, tag="stg")
                        quant_stage_b(nat, gmax, stg[:], nc.gpsimd)
                        nc.sync.dma_start(
                            out=cc_in[h][:, :, sl * 128:(sl + 1) * 128],
                            in_=stg[:],
                        )
                for h in range(NCH):
                    nc.gpsimd.collective_compute(
                        "AllGather",
                        ALU.bypass,
                        ins=[cc_in[h][:, :, :]],
                        outs=[cc_out[h][:, :, :]],
                        replica_groups=REPLICA_GROUPS,
                    )
                for h in range(NCH):
                    for nj in range(NSL):
                        nc.scalar.dma_start(
                            out=wqT[nj][:, h * CHT:(h + 1) * CHT, :],
                            in_=cc_out[h][nj * 128:(nj + 1) * 128, :, :],
                        )
            else:
                WB = 4
                for h in range(NCH):
                    for s0 in range(0, NS, WB):
                        for s in range(s0, min(NS, s0 + WB)):
                            nat, gmax = quant_stage_a(
                                w[s * 128:(s + 1) * 128,
                                  h * CH:(h + 1) * CH])
                            nj, no = divmod(s * 128, 512)
                            quant_stage_b(
                                nat, gmax,
                                wqT[nj][:, h * CHT:(h + 1) * CHT,
                                        no:no + 128],
                                nc.gpsimd,
                            )

            # ---- X phase: per m-strip quantize (2 strips ahead),
            # matmul, evict.  Steady-state x quant runs DVE-only so the
            # two vector engines never contend for SBUF ports.
            xqt_tiles = {}

            def quant_x_strip(m, add_eng):
                xqt = xqt_p.tile([128, NKT, 128], BF16, tag="xqt")
                xqt_tiles[m] = xqt
                quant_batch([
                    (x[m * 128:(m + 1) * 128, h * CH:(h + 1) * CH],
                     xqt[:, h * CHT:(h + 1) * CHT, :])
                    for h in range(NCH)
                ], add_eng)

            quant_x_strip(0, nc.vector)
            quant_x_strip(1, nc.vector)

            for m in range(MS):
                if m + 2 < MS and (m + 2) not in xqt_tiles:
                    quant_x_strip(m + 2, nc.vector)
                xqt = xqt_tiles[m]
                psum = psum_p.tile([128, SN], F32, tag="psum")
                for nj in range(NSL):
                    n0 = nj * 512
                    n1 = min(SN, n0 + 512)
                    # seed PSUM with the (doubled) bias via a K=1 matmul
                    nc.tensor.matmul(
                        psum[:, n0:n1],
                        ones[:],
                        bias2b[:, n0:n1],
                        start=True,
                        stop=False,
                    )
                for kt in range(NKT):
                    for nj in range(NSL):
                        n0 = nj * 512
                        n1 = min(SN, n0 + 512)
                        nc.tensor.matmul(
                            psum[:, n0:n1],
                            xqt[:, kt, :],
                            wqT[nj][:, kt, :n1 - n0],
                            start=False,
                            stop=(kt == NKT - 1),
                        )
                outt = out_p.tile([128, SN], F32, tag="outt")
                nc.scalar.copy(out=outt[:], in_=psum[:])
                nc.sync.dma_start(
                    out=o[m * 128:(m + 1) * 128, :], in_=outt[:]
                )

    nc.compile()
    return nc


_NC_CACHE = {}


def _get_nc(key=("full",)):
    if key not in _NC_CACHE:
        if key == ("full",):
            _NC_CACHE[key] = build_bass()
        else:
            _NC_CACHE[key] = build_bass(*key)
    return _NC_CACHE[key]


def kernel(input, weight, bias):
    input = np.ascontiguousarray(input, dtype=np.float32)
    weight = np.ascontiguousarray(weight, dtype=np.float32)
    bias = np.ascontiguousarray(bias, dtype=np.float32)

    nc = _get_nc()
    b2_full = bias * np.float32(2.0)

    in_maps = []
    for c in range(8):
        bi, ni = divmod(c, NCOL)
        w0 = ni * SN_FULL + bi * WP_ROWS
        in_maps.append({
            "x": input[bi * SM_FULL:(bi + 1) * SM_FULL, :],
            "wp": weight[w0:w0 + WP_ROWS, :],
            "b2": b2_full[ni * SN_FULL:(ni + 1) * SN_FULL],
        })

    trace = bool(int(os.environ.get("BFP_TRACE", "0")))
    res = run_bass_kernel_spmd(
        nc, in_maps, core_ids=list(range(8)), trace=trace,
    )
    kernel.last_results = res

    out = np.empty((B_FULL, OUT_FULL), dtype=np.float32)
    for c in range(8):
        bi, ni = divmod(c, NCOL)
        out[bi * SM_FULL:(bi + 1) * SM_FULL,
            ni * SN_FULL:(ni + 1) * SN_FULL] = res.results[c]["o"]
    return out


def bench(ins, iters=6):
    """Wall-clock timing of the jitted 8-core kernel (axon PJRT
    round-trip dominates; trace path gives true device time)."""
    import time
    import jax

    t0 = time.perf_counter()
    out = kernel(**ins)
    dt = time.perf_counter() - t0
    print("bench[wall incl gather]: %.3f ms" % (dt * 1e3))
    return max(1, int(dt * 1e9))


if __name__ == "__main__":
    import sys
    mode = sys.argv[1] if len(sys.argv) > 1 else "sim"
    if mode == "sim":
        # quick numerical validation in CoreSim on a small config
        # (ag=False: single-core local-W path exercises the same quant
        # math and matmul structure)
        from concourse.bass_interp import CoreSim
        SM, SN, K, CH = 256, 1024, 512, 256
        nc = build_bass(SM, SN, K, CH, ag=False)
        rng = np.random.default_rng(0)
        xin = rng.standard_normal((SM, K), dtype=np.float32)
        win = rng.uniform(-0.1, 0.1, (SN, K)).astype(np.float32)
        bin_ = rng.uniform(-0.1, 0.1, SN).astype(np.float32)

        sim = CoreSim(nc)
        sim.tensor("x")[:] = xin
        sim.tensor("w")[:] = win
        sim.tensor("b2")[:] = bin_ * 2.0
        sim.simulate(check_with_hw=False)
        got = np.array(sim.tensor("o"))

        def bfpq(v):
            g = v.reshape(v.shape[0], -1, GROUP).astype(np.float64)
            ma = np.abs(g).max(axis=-1, keepdims=True)
            e = np.floor(np.log2(np.where(ma > 0, ma, 1.0)))
            st = np.exp2(e - 6)
            qq = np.clip(np.round(g / st), -127, 127) * st
            return np.where(ma > 0, qq, 0.0).reshape(v.shape)

        exp = bfpq(xin) @ bfpq(win).T + 2.0 * bin_.astype(np.float64)
        err = np.abs(got.astype(np.float64) - exp)
        rel = err.max() / np.abs(exp).max()
        print("max abs err:", err.max(), "rel:", rel)
        assert rel < 1e-3, "numerical mismatch"
        print("SIM PASS")
    elif mode == "hw":
        import reference
        ins = {k: np.asarray(v) for k, v in reference.setup_inputs().items()}
        outp = kernel(**ins)
        print("out", outp.shape, outp.dtype)
